# revision 3
# baseline (speedup 1.0000x reference)
"""Binarize kernel for Trainium2, 8-core data-parallel, dual-queue DMA.

out[b, f] = 1.0 if (medians[f] > 0) and (x[b, f] >= medians[f]) else 0.0

Sharding: pure data parallel - x is split row-wise across the 8 NeuronCores
(2048 rows each); the 4096-entry medians vector is replicated.

Per-core device kernel (raw bass). The problem is pure HBM streaming
(32 MiB in + 32 MiB out per core, ~358 GB/s per-core limit), so the whole
design is about DMA-queue structure:
  * Both hardware DGE queues (SP + ACT) carry the bulk data, globally
    direction-phased: the 2048x4096 slice is cut into 16 tiles of
    [128, 4096] (2 MiB); each 10-tile phase loads 20 MiB (SP the first 5
    tiles, ACT the next 5, concurrently), then stores the 20 MiB of
    results. Cross-queue semaphore barriers keep the HBM bus
    single-direction during each burst. A single queue tops out ~344 GB/s;
    the dual direction-phased layout benches ~5-7 us/pass faster and
    matches a pure-DMA memcpy of the same footprint (~356 GB/s).
  * 10 SBUF slots (160 KiB/partition + 32 KiB for the medians) rather than
    8: longer single-direction bursts, 3.2 instead of 4 read/write
    turnarounds per 64 MiB.
  * DVE runs the compare in load-completion order (SP/ACT interleaved):
    xt = (xt >= mprime) in place, one exact fp32 compare per element, where
    mprime[f] = medians[f] if medians[f] > 0 else 3e38. No arithmetic
    rounding anywhere; the compare engine has 2x headroom over DMA and
    fully hides behind the loads.
  * The medians prep (16 KB load, two DVE ops on partition 0, log2
    doubling copies + fan-out to all 128 partitions, SBUF->SBUF) rides the
    gpsimd SWDGE queue so both data queues stream x from t=0.

Raw bass instead of the Tile framework because walrus codegen allows only a
single sync-wait command on a compute instruction; all waits here are
standalone queue commands. Per-slot load/store semaphore pairs make count
thresholds race-free even though DMA completions across slots may reorder.

reps > 1 re-runs the identical pipeline inside one NEFF (slope-based HW
timing); the output is unchanged.
"""

import contextlib
from collections import defaultdict

import numpy as np

import concourse.bass as bass
import concourse.mybir as mybir
from concourse.bass_utils import run_bass_kernel_spmd

N_CORES = 8
B_FULL = 16384
F = 4096
ROWS = B_FULL // N_CORES  # 2048 rows per core
P = 128
N_TILES = ROWS // P  # 16 tiles of [128, 4096] = 2 MiB
NSLOT = 10  # SBUF x-tile slots: SP uses 0-4, ACT 5-9
_BIG = 3.0e38  # pushes the compare threshold above any finite fp32 input


def _build_nc(reps: int = 1) -> bass.Bass:
    nc = bass.Bass()
    dt = mybir.dt.float32
    x = nc.dram_tensor("x", [ROWS, F], dt, kind="ExternalInput")
    med = nc.dram_tensor("med", [F], dt, kind="ExternalInput")
    out = nc.dram_tensor("out", [ROWS, F], dt, kind="ExternalOutput")
    x_t = x.rearrange("(n p) f -> n p f", p=P)
    o_t = out.rearrange("(n p) f -> n p f", p=P)

    # Global schedule: chunk the 16*reps-tile stream into 10-tile direction
    # phases; each phase SP takes the first ceil(n/2) tiles, ACT the rest.
    stream = [t % N_TILES for t in range(N_TILES * reps)]
    phases = []
    for i in range(0, len(stream), NSLOT):
        chunk = stream[i : i + NSLOT]
        h = (len(chunk) + 1) // 2
        phases.append((chunk[:h], chunk[h:]))

    sp_slots = [0, 1, 2, 3, 4]
    act_slots = [5, 6, 7, 8, 9]

    # Per-phase (tile, slot, use-index) schedules plus the DVE compare
    # sequence (SP/ACT interleaved = load-completion order).
    use_count = defaultdict(int)
    sp_sched, act_sched, dve_seq = [], [], []
    for sp_tiles, act_tiles in phases:
        sp_e, act_e = [], []
        for j, t in enumerate(sp_tiles):
            s = sp_slots[j]
            sp_e.append((t, s, use_count[s]))
            use_count[s] += 1
        for j, t in enumerate(act_tiles):
            s = act_slots[j]
            act_e.append((t, s, use_count[s]))
            use_count[s] += 1
        sp_sched.append(sp_e)
        act_sched.append(act_e)
        order = []
        for k in range(max(len(sp_e), len(act_e))):
            if k < len(sp_e):
                order.append(sp_e[k])
            if k < len(act_e):
                order.append(act_e[k])
        dve_seq.append(order)

    # (phase, slot) -> global DVE compare index; s_dve reads idx+3 once the
    # compare has retired (+2 is the mprime prep).
    dve_pos = {}
    gi = 0
    for ph, order in enumerate(dve_seq):
        for (_t, s, _u) in order:
            dve_pos[(ph, s)] = gi
            gi += 1

    with contextlib.ExitStack() as ctx:
        m_b = ctx.enter_context(nc.sbuf_tensor("m_b", [1, F], dt))
        mprime = ctx.enter_context(nc.sbuf_tensor("mprime", [P, F], dt))
        xt = ctx.enter_context(nc.sbuf_tensor("xt", [P, NSLOT, F], dt))
        s_med = ctx.enter_context(nc.semaphore("s_med"))
        s_fan = ctx.enter_context(nc.semaphore("s_fan"))
        s_ld = [
            ctx.enter_context(nc.semaphore(f"s_ld{s}")) for s in range(NSLOT)
        ]
        s_st = [
            ctx.enter_context(nc.semaphore(f"s_st{s}")) for s in range(NSLOT)
        ]
        s_dve = ctx.enter_context(nc.semaphore("s_dve"))
        block = ctx.enter_context(nc.Block())

        def queue_prog(eng, sched, other_sched):
            for ph, entries in enumerate(sched):
                if not entries:
                    continue
                for (t, s, u) in entries:
                    if u:
                        # slot reuse: its previous store must have drained
                        eng.wait_ge(s_st[s], 16 * u)
                    eng.dma_start(out=xt[:, s], in_=x_t[t]).then_inc(
                        s_ld[s], 16
                    )
                # direction barrier: the other queue's loads are also done
                if other_sched[ph]:
                    (_t, os_, ou) = other_sched[ph][-1]
                    eng.wait_ge(s_ld[os_], 16 * (ou + 1))
                for (t, s, u) in entries:
                    eng.wait_ge(s_dve, dve_pos[(ph, s)] + 3)
                    eng.dma_start(out=o_t[t], in_=xt[:, s]).then_inc(
                        s_st[s], 16
                    )
                if ph < len(sched) - 1 and other_sched[ph]:
                    # direction barrier before the next load burst
                    (_t, os_, ou) = other_sched[ph][-1]
                    eng.wait_ge(s_st[os_], 16 * (ou + 1))
            # all stores landed before the NEFF retires
            for s in sorted(set(s for e in sched for (_t, s, _u) in e)):
                eng.wait_ge(s_st[s], 16 * use_count[s])

        @block.sync
        def _(sync):
            queue_prog(sync, sp_sched, act_sched)

        @block.scalar
        def _(scalar):
            queue_prog(scalar, act_sched, sp_sched)

        @block.gpsimd
        def _(gpsimd):
            # medians setup on the SWDGE queue so both HWDGE data queues
            # stream x from t=0: 16 KB load -> partition 0, then after the
            # DVE prep, log2 doubling copies to 16 partitions + 7 concurrent
            # fan-out copies (SBUF->SBUF, no HBM traffic)
            gpsimd.dma_start(out=m_b[:1, :], in_=med[None, :]).then_inc(
                s_med, 16
            )
            gpsimd.wait_ge(s_dve, 2)
            k, chain = 1, 0
            while k < 16:
                gpsimd.dma_start(
                    out=mprime[k : 2 * k, :], in_=mprime[:k, :]
                ).then_inc(s_fan, 16)
                chain += 1
                gpsimd.wait_ge(s_fan, 16 * chain)
                k *= 2
            for j in range(1, 8):
                gpsimd.dma_start(
                    out=mprime[16 * j : 16 * (j + 1), :], in_=mprime[:16, :]
                ).then_inc(s_fan, 16)

        @block.vector
        def _(vector):
            vector.wait_ge(s_med, 16)  # medians row present
            # mprime = (med <= 0) * BIG + med, on partition 0 only; sem
            # handshakes order the back-to-back DVE ops (same-engine RAW is
            # not implicit)
            nc.vector.tensor_scalar(
                out=mprime[:1, :],
                in0=m_b[:1, :],
                scalar1=0.0,
                scalar2=_BIG,
                op0=mybir.AluOpType.is_le,
                op1=mybir.AluOpType.mult,
            ).then_inc(s_dve, 1)
            vector.wait_ge(s_dve, 1)
            nc.vector.tensor_add(
                out=mprime[:1, :], in0=mprime[:1, :], in1=m_b[:1, :]
            ).then_inc(s_dve, 1)
            vector.wait_ge(s_fan, 16 * 11)  # 4 doubling + 7 fan-out copies
            for ph, order in enumerate(dve_seq):
                for (_t, s, u) in order:
                    vector.wait_ge(s_ld[s], 16 * (u + 1))
                    nc.vector.tensor_tensor(
                        out=xt[:, s],
                        in0=xt[:, s],
                        in1=mprime[:],
                        op=mybir.AluOpType.is_ge,
                    ).then_inc(s_dve, 1)

    return nc


_NC_CACHE: list[bass.Bass] = []


def _get_nc() -> bass.Bass:
    if not _NC_CACHE:
        _NC_CACHE.append(_build_nc(reps=1))
    return _NC_CACHE[0]


def kernel(x: np.ndarray, medians: np.ndarray) -> np.ndarray:
    x = np.ascontiguousarray(x, dtype=np.float32)
    medians = np.ascontiguousarray(medians, dtype=np.float32)
    assert x.shape == (B_FULL, F), x.shape
    assert medians.shape == (F,), medians.shape

    nc = _get_nc()
    in_maps = [
        {"x": x[c * ROWS : (c + 1) * ROWS], "med": medians}
        for c in range(N_CORES)
    ]
    res = run_bass_kernel_spmd(nc, in_maps, core_ids=list(range(N_CORES)))
    return np.concatenate(
        [res.results[c]["out"] for c in range(N_CORES)], axis=0
    )


# revision 5
# speedup vs baseline: 1.0455x; 1.0455x over previous
"""Binarize kernel for Trainium2, 8-core data-parallel, dual-queue DMA.

out[b, f] = 1.0 if (medians[f] > 0) and (x[b, f] >= medians[f]) else 0.0

Sharding: pure data parallel - x is split row-wise across the 8 NeuronCores
(2048 rows each); the 4096-entry medians vector is replicated.

Per-core device kernel (raw bass). The problem is pure HBM streaming
(32 MiB in + 32 MiB out per core, ~358 GB/s per-core limit), so the whole
design is about DMA-queue structure:
  * Both hardware DGE queues (SP + ACT) carry the bulk data, globally
    direction-phased: the 2048x4096 slice is cut into 16 tiles of
    [128, 4096] (2 MiB); each 10-tile phase loads 20 MiB (SP the first 5
    tiles, ACT the next 5, concurrently), then stores the 20 MiB of
    results. Cross-queue semaphore barriers keep the HBM bus
    single-direction during each burst. A single queue tops out ~344 GB/s;
    the dual direction-phased layout benches ~5-7 us/pass faster and
    matches a pure-DMA memcpy of the same footprint (~356 GB/s).
  * 10 SBUF slots (160 KiB/partition + 32 KiB for the medians) rather than
    8: longer single-direction bursts, 3.2 instead of 4 read/write
    turnarounds per 64 MiB.
  * DVE runs the compare in load-completion order (SP/ACT interleaved):
    xt = (xt >= mprime) in place, one exact fp32 compare per element, where
    mprime[f] = medians[f] if medians[f] > 0 else 3e38. No arithmetic
    rounding anywhere; the compare engine has 2x headroom over DMA and
    fully hides behind the loads.
  * The medians prep (16 KB load, two DVE ops on partition 0, log2
    doubling copies + fan-out to all 128 partitions, SBUF->SBUF) rides the
    gpsimd SWDGE queue so both data queues stream x from t=0.

Raw bass instead of the Tile framework because walrus codegen allows only a
single sync-wait command on a compute instruction; all waits here are
standalone queue commands. Per-slot load semaphores keep the compare waits
race-free even though DMA completions across slots may reorder; stores
count into one shared semaphore per queue, so slot-reuse and direction
barriers are single batched waits (queue-side waits cost real time on the
DGE, so the schedule uses ~5 per phase instead of ~12).

reps > 1 re-runs the identical pipeline inside one NEFF (slope-based HW
timing); the output is unchanged.
"""

import contextlib
from collections import defaultdict

import numpy as np

import concourse.bass as bass
import concourse.mybir as mybir
from concourse.bass_utils import run_bass_kernel_spmd

N_CORES = 8
B_FULL = 16384
F = 4096
ROWS = B_FULL // N_CORES  # 2048 rows per core
P = 128
N_TILES = ROWS // P  # 16 tiles of [128, 4096] = 2 MiB
NSLOT = 10  # SBUF x-tile slots: SP uses 0-4, ACT 5-9
_BIG = 3.0e38  # pushes the compare threshold above any finite fp32 input


def _build_nc(reps: int = 1) -> bass.Bass:
    nc = bass.Bass()
    dt = mybir.dt.float32
    x = nc.dram_tensor("x", [ROWS, F], dt, kind="ExternalInput")
    med = nc.dram_tensor("med", [F], dt, kind="ExternalInput")
    out = nc.dram_tensor("out", [ROWS, F], dt, kind="ExternalOutput")
    x_t = x.rearrange("(n p) f -> n p f", p=P)
    o_t = out.rearrange("(n p) f -> n p f", p=P)

    # Global schedule: chunk the 16*reps-tile stream into 10-tile direction
    # phases; each phase SP takes the first ceil(n/2) tiles, ACT the rest.
    stream = [t % N_TILES for t in range(N_TILES * reps)]
    phases = []
    for i in range(0, len(stream), NSLOT):
        chunk = stream[i : i + NSLOT]
        h = (len(chunk) + 1) // 2
        phases.append((chunk[:h], chunk[h:]))

    sp_slots = [0, 1, 2, 3, 4]
    act_slots = [5, 6, 7, 8, 9]

    # Per-phase (tile, slot, use-index) schedules plus the DVE compare
    # sequence (SP/ACT interleaved = load-completion order).
    use_count = defaultdict(int)
    sp_sched, act_sched, dve_seq = [], [], []
    for sp_tiles, act_tiles in phases:
        sp_e, act_e = [], []
        for j, t in enumerate(sp_tiles):
            s = sp_slots[j]
            sp_e.append((t, s, use_count[s]))
            use_count[s] += 1
        for j, t in enumerate(act_tiles):
            s = act_slots[j]
            act_e.append((t, s, use_count[s]))
            use_count[s] += 1
        sp_sched.append(sp_e)
        act_sched.append(act_e)
        order = []
        for k in range(max(len(sp_e), len(act_e))):
            if k < len(sp_e):
                order.append(sp_e[k])
            if k < len(act_e):
                order.append(act_e[k])
        dve_seq.append(order)

    # (phase, slot) -> global DVE compare index; s_dve reads idx+3 once the
    # compare has retired (+2 is the mprime prep).
    dve_pos = {}
    gi = 0
    for ph, order in enumerate(dve_seq):
        for (_t, s, _u) in order:
            dve_pos[(ph, s)] = gi
            gi += 1

    with contextlib.ExitStack() as ctx:
        m_b = ctx.enter_context(nc.sbuf_tensor("m_b", [1, F], dt))
        mprime = ctx.enter_context(nc.sbuf_tensor("mprime", [P, F], dt))
        xt = ctx.enter_context(nc.sbuf_tensor("xt", [P, NSLOT, F], dt))
        s_med = ctx.enter_context(nc.semaphore("s_med"))
        s_fan = ctx.enter_context(nc.semaphore("s_fan"))
        s_ld = [
            ctx.enter_context(nc.semaphore(f"s_ld{s}")) for s in range(NSLOT)
        ]
        s_stq = [
            ctx.enter_context(nc.semaphore(f"s_stq{q}")) for q in range(2)
        ]
        s_dve = ctx.enter_context(nc.semaphore("s_dve"))
        block = ctx.enter_context(nc.Block())

        # cumulative store count of a queue through phase ph, for shared
        # store-counter thresholds (queue waits are batched: one reuse wait
        # and two compare waits per phase instead of one per tile)
        def cum_stores(sched, ph):
            return sum(len(sched[i]) for i in range(ph + 1))

        def queue_prog(eng, q, sched, other_q, other_sched):
            for ph, entries in enumerate(sched):
                if not entries:
                    continue
                if ph >= 1:
                    # all own stores through phase ph-1 drained -> every own
                    # slot is reusable
                    eng.wait_ge(s_stq[q], 16 * cum_stores(sched, ph - 1))
                for (t, s, _u) in entries:
                    eng.dma_start(out=xt[:, s], in_=x_t[t]).then_inc(
                        s_ld[s], 16
                    )
                # direction barrier: the other queue's loads are also done
                if other_sched[ph]:
                    (_t, os_, ou) = other_sched[ph][-1]
                    eng.wait_ge(s_ld[os_], 16 * (ou + 1))
                # batched compare waits: one covering all but the last tile
                # (their compares retire well before the load burst ends),
                # then one for the last
                if len(entries) > 1:
                    head_max = max(
                        dve_pos[(ph, s)] for (_t, s, _u) in entries[:-1]
                    )
                    eng.wait_ge(s_dve, head_max + 3)
                    for (t, s, _u) in entries[:-1]:
                        eng.dma_start(out=o_t[t], in_=xt[:, s]).then_inc(
                            s_stq[q], 16
                        )
                (t, s, _u) = entries[-1]
                eng.wait_ge(s_dve, dve_pos[(ph, s)] + 3)
                eng.dma_start(out=o_t[t], in_=xt[:, s]).then_inc(
                    s_stq[q], 16
                )
                if ph < len(sched) - 1 and other_sched[ph]:
                    # direction barrier before the next load burst
                    eng.wait_ge(
                        s_stq[other_q], 16 * cum_stores(other_sched, ph)
                    )
            # all stores landed before the NEFF retires
            eng.wait_ge(s_stq[q], 16 * cum_stores(sched, len(sched) - 1))

        @block.sync
        def _(sync):
            queue_prog(sync, 0, sp_sched, 1, act_sched)

        @block.scalar
        def _(scalar):
            queue_prog(scalar, 1, act_sched, 0, sp_sched)

        @block.gpsimd
        def _(gpsimd):
            # medians setup on the SWDGE queue so both HWDGE data queues
            # stream x from t=0: 16 KB load -> partition 0, then after the
            # DVE prep, log2 doubling copies to 16 partitions + 7 concurrent
            # fan-out copies (SBUF->SBUF, no HBM traffic)
            gpsimd.dma_start(out=m_b[:1, :], in_=med[None, :]).then_inc(
                s_med, 16
            )
            gpsimd.wait_ge(s_dve, 2)
            k, chain = 1, 0
            while k < 16:
                gpsimd.dma_start(
                    out=mprime[k : 2 * k, :], in_=mprime[:k, :]
                ).then_inc(s_fan, 16)
                chain += 1
                gpsimd.wait_ge(s_fan, 16 * chain)
                k *= 2
            for j in range(1, 8):
                gpsimd.dma_start(
                    out=mprime[16 * j : 16 * (j + 1), :], in_=mprime[:16, :]
                ).then_inc(s_fan, 16)

        @block.vector
        def _(vector):
            vector.wait_ge(s_med, 16)  # medians row present
            # mprime = (med <= 0) * BIG + med, on partition 0 only; sem
            # handshakes order the back-to-back DVE ops (same-engine RAW is
            # not implicit)
            nc.vector.tensor_scalar(
                out=mprime[:1, :],
                in0=m_b[:1, :],
                scalar1=0.0,
                scalar2=_BIG,
                op0=mybir.AluOpType.is_le,
                op1=mybir.AluOpType.mult,
            ).then_inc(s_dve, 1)
            vector.wait_ge(s_dve, 1)
            nc.vector.tensor_add(
                out=mprime[:1, :], in0=mprime[:1, :], in1=m_b[:1, :]
            ).then_inc(s_dve, 1)
            vector.wait_ge(s_fan, 16 * 11)  # 4 doubling + 7 fan-out copies
            for ph, order in enumerate(dve_seq):
                for (_t, s, u) in order:
                    vector.wait_ge(s_ld[s], 16 * (u + 1))
                    nc.vector.tensor_tensor(
                        out=xt[:, s],
                        in0=xt[:, s],
                        in1=mprime[:],
                        op=mybir.AluOpType.is_ge,
                    ).then_inc(s_dve, 1)

    return nc


_NC_CACHE: list[bass.Bass] = []


def _get_nc() -> bass.Bass:
    if not _NC_CACHE:
        _NC_CACHE.append(_build_nc(reps=1))
    return _NC_CACHE[0]


def kernel(x: np.ndarray, medians: np.ndarray) -> np.ndarray:
    x = np.ascontiguousarray(x, dtype=np.float32)
    medians = np.ascontiguousarray(medians, dtype=np.float32)
    assert x.shape == (B_FULL, F), x.shape
    assert medians.shape == (F,), medians.shape

    nc = _get_nc()
    in_maps = [
        {"x": x[c * ROWS : (c + 1) * ROWS], "med": medians}
        for c in range(N_CORES)
    ]
    res = run_bass_kernel_spmd(nc, in_maps, core_ids=list(range(N_CORES)))
    return np.concatenate(
        [res.results[c]["out"] for c in range(N_CORES)], axis=0
    )
